# revision 16
# baseline (speedup 1.0000x reference)
"""VQ codebook pairwise squared-euclidean distances on 8 trn2 NeuronCores.

out[n, u] = ||x_n||^2 + ||w_u||^2 - 2 * x_n . w_u
  inputs: [16384, 1024] f32, w: [4096, 1024] f32 -> out [16384, 4096] f32

Strategy: data-parallel shard of N across 8 cores (2048 rows each), W
replicated. Per core: fp8(e4m3) GEMM on the tensor engine in DoubleRow
perf mode (2 MACs/cell/cycle, fp32 PSUM accum; w is pre-scaled by 64 to
stay in e4m3's normal range, undone in the epilogue scale). The two
rank-1 terms are fused in the epilogue on ScalarE (per-partition bias:
-2/64*psum + x_sq) and VectorE (+w_sq broadcast tile). Host preps fp8
K-packed layouts so every input load is one contiguous 1 MB DMA and no
on-device transposes are needed. Output stores alternate between the
two HWDGE rings (sync/scalar) to sustain the fp8-rate store stream.
"""

import sys

import ml_dtypes
import numpy as np

if "/opt/trn_rl_repo" not in sys.path:
    sys.path.insert(0, "/opt/trn_rl_repo")

N, D, U = 16384, 1024, 4096
NCORES = 8
NS = N // NCORES  # 2048 rows per core
P = 128
KK = D // 256  # 4 DoubleRow super k-tiles (256 contraction each)
MT = NS // P  # 16 m-tiles per core
UT = U // 512  # 8 u-tiles of 512 cols
MC = 4  # m-tiles per xt load chunk (512 cols)
WSCALE = 64.0  # w pre-scale into e4m3 normal range (power of 2: exact)

_cache = {}


def _build():
    import concourse.bacc as bacc
    import concourse.mybir as mybir
    import concourse.tile as tile

    dt = mybir.dt
    AF = mybir.ActivationFunctionType
    ALU = mybir.AluOpType
    DR = mybir.MatmulPerfMode.DoubleRow

    nc = bacc.Bacc("TRN2", debug=False, target_bir_lowering=False)
    # Host-pre-packed fp8 inputs: block b holds [p=128, kk, i, c] where the
    # contraction index is d = kk*256 + i*128 + p (DoubleRow packs pairs
    # (p, i) into one PE cell). Each block is one contiguous 1 MB DMA.
    xt_d = nc.dram_tensor("xt", [MT // MC, P, KK, 2, 512], dt.float8e4, kind="ExternalInput")
    wt_d = nc.dram_tensor("wt", [UT, P, KK, 2, 512], dt.float8e4, kind="ExternalInput")
    xsq_d = nc.dram_tensor("xsq", [P, MT], dt.float32, kind="ExternalInput")
    wsq_d = nc.dram_tensor("wsq", [P, U], dt.bfloat16, kind="ExternalInput")
    out_d = nc.dram_tensor("out", [NS, U], dt.float32, kind="ExternalOutput")

    with tile.TileContext(nc) as tc:
        with (
            tc.tile_pool(name="const", bufs=1) as cpool,
            tc.tile_pool(name="psum", bufs=4, space="PSUM") as psum_pool,
            tc.tile_pool(name="outp", bufs=24) as out_pool,
        ):
            # Small epilogue constants ride the scalar HWDGE ring, which is
            # otherwise idle until outputs start.
            xsq_sb = cpool.tile([P, MT], dt.float32, tag="xsq")
            nc.scalar.dma_start(xsq_sb[:], xsq_d[:, :])
            wsq_sb = cpool.tile([P, U], dt.bfloat16, tag="wsq")
            nc.scalar.dma_start(wsq_sb[:], wsq_d[:, :])

            xt_sb = {}
            wt_sb = {}

            def load_xt(mc):
                t = cpool.tile([P, KK, 2, 512], dt.float8e4, tag=f"xt_{mc}")
                nc.sync.dma_start(t[:], xt_d[mc])
                xt_sb[mc] = t

            def load_wt(u):
                t = cpool.tile([P, KK, 2, 512], dt.float8e4, tag=f"wt_{u}")
                nc.sync.dma_start(t[:], wt_d[u])
                wt_sb[u] = t

            # DMA program order = consumption priority on the sync ring.
            load_wt(0)
            load_xt(0)
            load_xt(1)
            load_xt(2)
            load_xt(3)
            for u in range(1, UT):
                load_wt(u)

            # PE warm-up: dummy matmuls run while the first input DMAs are
            # still in flight, so the HAM clock-gate is at full rate (and the
            # PE pipeline primed) by the time real matmuls start.
            warm = cpool.tile([P, 512], dt.float8e4, tag="warm")
            nc.vector.memset(warm[:], 0.0)
            warm_ps = psum_pool.tile([P, 512], dt.float32, tag="warm_ps")
            for _ in range(24):
                nc.tensor.matmul(
                    warm_ps[:], warm[:, 0:P], warm[:], start=True, stop=True
                )

            for u in range(UT):
                for m in range(MT):
                    mc, mo = divmod(m, MC)
                    ps = psum_pool.tile([P, 512], dt.float32, tag="ps")
                    for kk in range(KK):
                        nc.tensor.matmul(
                            ps[:],
                            xt_sb[mc][:, kk, :, mo * P : (mo + 1) * P],
                            wt_sb[u][:, kk, :, :],
                            start=(kk == 0),
                            stop=(kk == KK - 1),
                            perf_mode=DR,
                        )
                    ot = out_pool.tile([P, 512], dt.float32, tag="ot")
                    nc.scalar.activation(
                        ot[:],
                        ps[:],
                        AF.Identity,
                        bias=xsq_sb[:, m : m + 1],
                        scale=-2.0 / WSCALE,
                    )
                    nc.vector.tensor_tensor(
                        ot[:], ot[:], wsq_sb[:, u * 512 : (u + 1) * 512], ALU.add
                    )
                    # Alternate output stores across the two HWDGE rings so
                    # the fp8-rate store stream isn't ring-limited.
                    eng = nc.sync if (u * MT + m) % 2 == 0 else nc.scalar
                    eng.dma_start(
                        out_d[m * P : (m + 1) * P, u * 512 : (u + 1) * 512], ot[:]
                    )
    nc.compile()
    return nc


def _get_nc():
    if "nc" not in _cache:
        _cache["nc"] = _build()
    return _cache["nc"]


def _prep_inputs(inputs, w):
    f8 = ml_dtypes.float8_e4m3
    x = np.ascontiguousarray(np.asarray(inputs, dtype=np.float32))
    wf = np.ascontiguousarray(np.asarray(w, dtype=np.float32))

    # [u, p, kk, i, c]: element = w[u*512 + c, kk*256 + i*128 + p] * WSCALE
    wt = np.ascontiguousarray(
        (wf * WSCALE).astype(f8).reshape(UT, 512, KK, 2, P).transpose(0, 4, 2, 3, 1)
    )
    w_sq = (wf.astype(np.float64) ** 2).sum(-1).astype(ml_dtypes.bfloat16)  # [U]
    wsq_bc = np.ascontiguousarray(np.broadcast_to(w_sq[None, :], (P, U)))
    x_sq = (x.astype(np.float64) ** 2).sum(-1).astype(np.float32)  # [N]

    in_maps = []
    for c in range(NCORES):
        xs = x[c * NS : (c + 1) * NS]
        # [mc, p, kk, i, c]: element = x[n = mc*512 + col, d = kk*256 + i*128 + p]
        xt = np.ascontiguousarray(
            xs.astype(f8).reshape(MT // MC, 512, KK, 2, P).transpose(0, 4, 2, 3, 1)
        )
        xsq_t = np.ascontiguousarray(
            x_sq[c * NS : (c + 1) * NS].reshape(MT, P).T
        )  # [P, MT]
        in_maps.append({"xt": xt, "wt": wt, "xsq": xsq_t, "wsq": wsq_bc})
    return in_maps


def run(inputs, w, trace=False, **trace_kwargs):
    """Run on hardware; returns (out, BassKernelResults)."""
    from concourse.bass_utils import run_bass_kernel_spmd

    nc = _get_nc()
    in_maps = _prep_inputs(inputs, w)
    res = run_bass_kernel_spmd(
        nc, in_maps, list(range(NCORES)), trace=trace, **trace_kwargs
    )
    out = np.concatenate([r["out"] for r in res.results], axis=0)
    return np.ascontiguousarray(out, dtype=np.float32), res


def kernel(inputs, w):
    out, _ = run(inputs, w)
    return out


# revision 20
# speedup vs baseline: 1.2146x; 1.2146x over previous
"""VQ codebook pairwise squared-euclidean distances on 8 trn2 NeuronCores.

out[n, u] = ||x_n||^2 + ||w_u||^2 - 2 * x_n . w_u
  inputs: [16384, 1024] f32, w: [4096, 1024] f32 -> out [16384, 4096] f32

Strategy: data-parallel shard of N across 8 cores (2048 rows each), W
replicated. Per core: fp8(e4m3) GEMM on the tensor engine in DoubleRow
perf mode (2 MACs/cell/cycle, fp32 PSUM accum; w is pre-scaled by 64 to
stay in e4m3's normal range, undone in the epilogue scale). The two
rank-1 terms are fused in the epilogue on ScalarE (per-partition bias:
-2/64*psum + x_sq) and VectorE (+w_sq broadcast tile). Host preps fp8
K-packed layouts so every input load is one contiguous 1 MB DMA and no
on-device transposes are needed. Output stores alternate between the
two HWDGE rings (sync/scalar) to sustain the fp8-rate store stream.
"""

import sys

import ml_dtypes
import numpy as np

if "/opt/trn_rl_repo" not in sys.path:
    sys.path.insert(0, "/opt/trn_rl_repo")

N, D, U = 16384, 1024, 4096
NCORES = 8
NS = N // NCORES  # 2048 rows per core
P = 128
KK = D // 256  # 4 DoubleRow super k-tiles (256 contraction each)
MT = NS // P  # 16 m-tiles per core
UT = U // 512  # 8 u-tiles of 512 cols
MC = 4  # m-tiles per xt load chunk (512 cols)
WSCALE = 64.0  # w pre-scale into e4m3 normal range (power of 2: exact)

_cache = {}


def _build():
    import concourse.bacc as bacc
    import concourse.mybir as mybir
    import concourse.tile as tile

    dt = mybir.dt
    AF = mybir.ActivationFunctionType
    ALU = mybir.AluOpType
    DR = mybir.MatmulPerfMode.DoubleRow

    nc = bacc.Bacc("TRN2", debug=False, target_bir_lowering=False)
    # Host-pre-packed fp8 inputs: block b holds [p=128, kk, i, c] where the
    # contraction index is d = kk*256 + i*128 + p (DoubleRow packs pairs
    # (p, i) into one PE cell). Each block is one contiguous 1 MB DMA.
    xt_d = nc.dram_tensor("xt", [MT // MC, P, KK, 2, 512], dt.float8e4, kind="ExternalInput")
    wt_d = nc.dram_tensor("wt", [UT, P, KK, 2, 512], dt.float8e4, kind="ExternalInput")
    xsq_d = nc.dram_tensor("xsq", [P, MT], dt.float32, kind="ExternalInput")
    wsq_d = nc.dram_tensor("wsq", [P, U], dt.bfloat16, kind="ExternalInput")
    out_d = nc.dram_tensor("out", [NS, U], dt.float32, kind="ExternalOutput")

    with tile.TileContext(nc) as tc:
        with (
            tc.tile_pool(name="const", bufs=1) as cpool,
            tc.tile_pool(name="psum", bufs=4, space="PSUM") as psum_pool,
            tc.tile_pool(name="outp", bufs=24) as out_pool,
        ):
            # Small epilogue constants ride the scalar HWDGE ring, which is
            # otherwise idle until outputs start.
            xsq_sb = cpool.tile([P, MT], dt.float32, tag="xsq")
            nc.scalar.dma_start(xsq_sb[:], xsq_d[:, :])
            wsq_sb = cpool.tile([P, U], dt.bfloat16, tag="wsq")
            nc.scalar.dma_start(wsq_sb[:], wsq_d[:, :])

            xt_sb = {}
            wt_sb = {}

            def load_xt(mc):
                t = cpool.tile([P, KK, 2, 512], dt.float8e4, tag=f"xt_{mc}")
                nc.sync.dma_start(t[:], xt_d[mc])
                xt_sb[mc] = t

            def load_wt(u):
                t = cpool.tile([P, KK, 2, 512], dt.float8e4, tag=f"wt_{u}")
                nc.sync.dma_start(t[:], wt_d[u])
                wt_sb[u] = t

            # DMA program order = consumption priority on the sync ring.
            load_wt(0)
            load_xt(0)
            load_xt(1)
            load_xt(2)
            load_xt(3)
            for u in range(1, UT):
                load_wt(u)

            for u in range(UT):
                for m in range(MT):
                    mc, mo = divmod(m, MC)
                    ps = psum_pool.tile([P, 512], dt.float32, tag="ps")
                    for kk in range(KK):
                        nc.tensor.matmul(
                            ps[:],
                            xt_sb[mc][:, kk, :, mo * P : (mo + 1) * P],
                            wt_sb[u][:, kk, :, :],
                            start=(kk == 0),
                            stop=(kk == KK - 1),
                            perf_mode=DR,
                        )
                    ot = out_pool.tile([P, 512], dt.float32, tag="ot")
                    nc.scalar.activation(
                        ot[:],
                        ps[:],
                        AF.Identity,
                        bias=xsq_sb[:, m : m + 1],
                        scale=-2.0 / WSCALE,
                    )
                    nc.vector.tensor_tensor(
                        ot[:], ot[:], wsq_sb[:, u * 512 : (u + 1) * 512], ALU.add
                    )
                    # Alternate output stores across the two HWDGE rings so
                    # the fp8-rate store stream isn't ring-limited.
                    eng = nc.sync if (u * MT + m) % 2 == 0 else nc.scalar
                    eng.dma_start(
                        out_d[m * P : (m + 1) * P, u * 512 : (u + 1) * 512], ot[:]
                    )
    nc.compile()
    return nc


def _get_nc():
    if "nc" not in _cache:
        _cache["nc"] = _build()
    return _cache["nc"]


def _prep_inputs(inputs, w):
    f8 = ml_dtypes.float8_e4m3
    x = np.ascontiguousarray(np.asarray(inputs, dtype=np.float32))
    wf = np.ascontiguousarray(np.asarray(w, dtype=np.float32))

    # [u, p, kk, i, c]: element = w[u*512 + c, kk*256 + i*128 + p] * WSCALE
    wt = np.ascontiguousarray(
        (wf * WSCALE).astype(f8).reshape(UT, 512, KK, 2, P).transpose(0, 4, 2, 3, 1)
    )
    w_sq = (wf.astype(np.float64) ** 2).sum(-1).astype(ml_dtypes.bfloat16)  # [U]
    wsq_bc = np.ascontiguousarray(np.broadcast_to(w_sq[None, :], (P, U)))
    x_sq = (x.astype(np.float64) ** 2).sum(-1).astype(np.float32)  # [N]

    in_maps = []
    for c in range(NCORES):
        xs = x[c * NS : (c + 1) * NS]
        # [mc, p, kk, i, c]: element = x[n = mc*512 + col, d = kk*256 + i*128 + p]
        xt = np.ascontiguousarray(
            xs.astype(f8).reshape(MT // MC, 512, KK, 2, P).transpose(0, 4, 2, 3, 1)
        )
        xsq_t = np.ascontiguousarray(
            x_sq[c * NS : (c + 1) * NS].reshape(MT, P).T
        )  # [P, MT]
        in_maps.append({"xt": xt, "wt": wt, "xsq": xsq_t, "wsq": wsq_bc})
    return in_maps


def run(inputs, w, trace=False, **trace_kwargs):
    """Run on hardware via concourse; returns (out, BassKernelResults)."""
    from concourse.bass_utils import run_bass_kernel_spmd

    nc = _get_nc()
    in_maps = _prep_inputs(inputs, w)
    res = run_bass_kernel_spmd(
        nc, in_maps, list(range(NCORES)), trace=trace, **trace_kwargs
    )
    out = np.concatenate([r["out"] for r in res.results], axis=0)
    return np.ascontiguousarray(out, dtype=np.float32), res


def _get_runner():
    """Cached jitted SPMD executable (mirrors bass2jax.run_bass_via_pjrt's
    multi-core branch) so repeat kernel() calls skip recompilation."""
    if "runner" in _cache:
        return _cache["runner"]
    import jax
    from concourse import bass2jax as b2j
    from concourse import mybir
    from jax.experimental.shard_map import shard_map
    from jax.sharding import Mesh, PartitionSpec

    nc = _get_nc()
    b2j.install_neuronx_cc_hook()
    partition_name = nc.partition_id_tensor.name if nc.partition_id_tensor else None
    in_names, out_names, out_avals, zero_shapes = [], [], [], []
    for alloc in nc.m.functions[0].allocations:
        if not isinstance(alloc, mybir.MemoryLocationSet):
            continue
        name = alloc.memorylocations[0].name
        if alloc.kind == "ExternalInput":
            if name != partition_name:
                in_names.append(name)
        elif alloc.kind == "ExternalOutput":
            out_names.append(name)
            shape, dtype = tuple(alloc.tensor_shape), mybir.dt.np(alloc.dtype)
            out_avals.append(jax.core.ShapedArray(shape, dtype))
            zero_shapes.append((shape, dtype))
    n_params, n_outs = len(in_names), len(out_names)
    all_in_names = in_names + out_names + ([partition_name] if partition_name else [])

    def _body(*args):
        operands = list(args)
        if partition_name is not None:
            operands.append(b2j.partition_id_tensor())
        return tuple(
            b2j._bass_exec_p.bind(
                *operands,
                out_avals=tuple(out_avals),
                in_names=tuple(all_in_names),
                out_names=tuple(out_names),
                lowering_input_output_aliases=(),
                sim_require_finite=True,
                sim_require_nnan=True,
                nc=nc,
            )
        )

    devices = jax.devices()[:NCORES]
    mesh = Mesh(np.asarray(devices), ("core",))
    sharded = jax.jit(
        shard_map(
            _body,
            mesh=mesh,
            in_specs=(PartitionSpec("core"),) * (n_params + n_outs),
            out_specs=(PartitionSpec("core"),) * n_outs,
            check_rep=False,
        ),
        donate_argnums=tuple(range(n_params, n_params + n_outs)),
        keep_unused=True,
    )

    # Donated output backing store, created device-side (the kernel writes
    # every element, so the zeros never cross the host<->device tunnel).
    import jax.numpy as jnp
    from jax.sharding import NamedSharding

    sharding = NamedSharding(mesh, PartitionSpec("core"))

    def zeros_maker(shape, dtype):
        return jax.jit(
            lambda: jnp.zeros((NCORES * shape[0], *shape[1:]), dtype),
            out_shardings=sharding,
        )

    makers = [zeros_maker(s, dt) for s, dt in zero_shapes]
    _cache["runner"] = (sharded, in_names, out_names, makers)
    return _cache["runner"]


def kernel(inputs, w):
    sharded, in_names, out_names, makers = _get_runner()
    in_maps = _prep_inputs(inputs, w)
    concat_in = [
        np.concatenate([m[name] for m in in_maps], axis=0) for name in in_names
    ]
    concat_zeros = [mk() for mk in makers]
    out_arrs = sharded(*concat_in, *concat_zeros)
    out = np.asarray(out_arrs[out_names.index("out")])
    return np.ascontiguousarray(out.reshape(N, U), dtype=np.float32)
